# revision 13
# baseline (speedup 1.0000x reference)
"""DistanceCentroidLoss on 8 Trainium2 NeuronCores (Bass/Tile), v3.

Data-parallel over N: each core processes NS=32768 rows (D=128, K=64).

Per block of 8 tiles (1024 rows):
  1. preinit (DVE): PSUM[row, (t,j)] = c2[j] + x2[row,t]
  2. xc: per tile one fp8 matmul accumulates -2 x.c_j onto it
     (lhsT = x-tile [128d, 128rows] fp8, rhs = cTm2 [128d, 64] fp8)
  3. ACT sqrt: dist f16 -> dd
  4. DVE: moh = dist*oh (2x-mode f16; oh itself is host-built and DMA'd
     per chunk into the interleaved ohmoh arena [128, T, 2, K])
  5. G (two blocks later): per tile lhsT = ohmoh[:, t] [128, 2, 64] (128-col
     weights, the fast ldweights shape), rhs = dd tile [128, 64]:
       G_t[s*64+k, j] += sum_n ohm[n,s,k] * dist[n,j]
     accumulated over all 256 tiles in one PSUM region [128, 64].
Host: G1 = G_t[0:64].T, G2 = G_t[64:128].T give per-cluster O (diag), W
(colsum-diag), A (diag of G2: moh*dist = d2_own at the label column). V comes
from a closed form (row-sums of d2 are linear in x2, c2, x.sum_j c_j),
evaluated with the device-effective fp8/f32 values so quantization matches.

Toolchain quirk: this walrus build rejects any instruction with more than one
semaphore wait. Single SWDGE lane, wait-carrying nops (PE pre-observes the
chunk DMA once per chunk), emission ordered so every instruction needs at
most one new cross-engine wait (vector clocks make Tile elide the rest),
plus a post-pass stripping redundant waits.
"""
import sys

sys.path.insert(0, '/opt/trn_rl_repo')

import numpy as np
import ml_dtypes

import concourse.bass as bass
import concourse.mybir as mybir
import concourse.tile_sem_assignment as _tsa
from concourse.tile import TileContext, add_dep_helper
from concourse.bass_utils import run_bass_kernel_spmd

f32 = mybir.dt.float32
bf16 = mybir.dt.bfloat16
f16 = mybir.dt.float16
f8 = mybir.dt.float8e4
E4M3 = ml_dtypes.float8_e4m3

N, D, K = 262144, 128, 64
NCORES = 8
NS = N // NCORES        # rows per core = 32768
T = NS // 128           # 256 tiles of 128 rows
TPB = 8                 # tiles per block
NB = T // TPB           # 32 blocks
TPC = 32                # tiles per DMA chunk
NCH = T // TPC          # 8 chunks
BPC = TPC // TPB        # blocks per chunk = 4
MARGIN = 5.0
CW = 528                # const pack: f32 words per partition

_cache = {}

_OWN_PREFIX = {
    "Activation": ("Activation_",),
    "DVE": ("DVE_",),
    "PE": ("PE_",),
    "Pool": ("Pool_",),
    "SP": ("SP_sequencer",),
}


def _is_own(eng, name):
    for p in _OWN_PREFIX.get(eng, ()):
        if name.startswith(p) and not name.startswith("barrier"):
            return True
    return False


def _strip_redundant_waits(nc):
    """Vector-clock transitive wait elision (see module docstring).

    Tile emits a sem wait for every cross-engine dep edge, but waiting on
    sem S >= v transitively guarantees everything the incrementing
    instruction had itself observed. Walk the lowered program order keeping
    a vector clock per engine (and per DMA ring); drop any wait already
    covered by the waiter's clock.
    """
    clocks = {}                  # proc -> {sem_name: max known-complete value}
    snaps = {}                   # sem_name -> {value: merged clock dict}
    counts = {}                  # sem_name -> current value

    def merge(dst, srcd):
        for k, v in srcd.items():
            if dst.get(k, -1) < v:
                dst[k] = v

    for blk in nc.m.functions[0].blocks:
        for inst in blk.instructions:
            eng = str(inst.engine).split(".")[-1]
            is_dma = "DMA" in type(inst).__name__ \
                or "DmaTranspose" in type(inst).__name__
            ck = clocks.setdefault(eng, {})
            si = inst.sync_info
            if si is not None and si.on_wait:
                keep = []
                for w in si.on_wait:
                    if w.sync_type != "semaphore" or w.wait_mode != "sem-ge-imm" \
                            or w.wait_reg is not None or w.wait_value is None \
                            or "barrier" in w.ant_name:
                        keep.append(w)
                        continue
                    nm, v = w.ant_name, w.wait_value
                    if ck.get(nm, -1) >= v:
                        continue            # already covered: drop
                    keep.append(w)
                    sn = snaps.get(nm, {}).get(v)
                    if sn is not None:
                        merge(ck, sn)
                    if ck.get(nm, -1) < v:
                        ck[nm] = v
                if len(keep) != len(si.on_wait):
                    si.on_wait = keep
                    inst.sync_info = si
            # completion clock of this instruction
            if is_dma:
                eff = dict(clocks.setdefault("DMA_RING", {}))
                merge(eff, ck)
            else:
                eff = ck
            if si is not None and si.on_update:
                for u in si.on_update:
                    if u.update_mode != "sem-inc" or u.update_value is None \
                            or "barrier" in u.ant_name:
                        continue
                    nm = u.ant_name
                    val = counts.get(nm, 0) + u.update_value
                    counts[nm] = val
                    snap = dict(eff)
                    snap[nm] = val
                    snaps.setdefault(nm, {})[val] = snap
                    if not is_dma:
                        # own-engine progress is known to itself
                        if ck.get(nm, -1) < val:
                            ck[nm] = val
            if is_dma:
                clocks["DMA_RING"] = eff
    return nc


def _build():
    # single completion-sem lane for SWDGE: all DMAs share DMASW0, so
    # consumers never carry waits on more than one DMA proc.
    _tsa.NUM_SWDGE_GLOBAL_SEMS = 1
    _tsa.NUM_HWDGE_SEMS = 1

    nc = bass.Bass()
    x8_in = nc.dram_tensor("x8", [128, NS], f8, kind="ExternalInput")
    oh_in = nc.dram_tensor("ohf", [128, T * K], f16, kind="ExternalInput")
    consts_in = nc.dram_tensor("consts", [128, CW], f32, kind="ExternalInput")
    out_G = nc.dram_tensor("out_G", [128, K], f32, kind="ExternalOutput")

    with TileContext(nc) as tc:
        with tc.tile_pool(name="single", bufs=1) as sb, \
             tc.tile_pool(name="ohp", bufs=3) as ohp, \
             tc.tile_pool(name="pxc", bufs=2, space="PSUM") as pxc, \
             tc.tile_pool(name="pacc", bufs=1, space="PSUM") as pacc:

            # ---- one packed constant DMA on the SWDGE lane ----
            consts_sb = sb.tile([128, CW], f32)
            cdma = nc.gpsimd.dma_start(out=consts_sb, in_=consts_in[:])
            # layout (f32 words): [0:16) ctf8 | [16:272) c2blk bf16[512] |
            # [272:528) x2cols f32[256]
            ctf8 = consts_sb[:, 0:16].bitcast(f8)              # [128,64] fp8
            c2blk = consts_sb[:, 16:272].bitcast(bf16)         # [128,512] bf16
            x2cols = consts_sb[:, 272:528]                     # [128,256] f32

            # ---- persistent per-chunk x arenas + dist store ----
            x8s = [sb.tile([128, TPC * 128], f8, name=f"x8{c}") for c in range(NCH)]
            dd = sb.tile([128, T, K], f16)            # 4 MB: dist only
            ohmoh = sb.tile([128, T, 2, K], f16)      # 8 MB: [oh | moh] per tile

            G_ps = pacc.tile([128, K], f32)

            casts = []
            sqrts = []
            blocks = []          # (b, ohm tile, moh inst)
            dve_tail = {"i": None}
            pe_tail = {"i": None}

            def dve_pin(i):
                # Tile's lowering reorders ready instructions within an engine
                # queue; the wait-elision scheme depends on DVE executing in
                # emission order, so chain every DVE op with nosync pins.
                if dve_tail["i"] is not None:
                    add_dep_helper(i.ins, dve_tail["i"].ins, sync=False,
                                   reason="dve order pin")
                dve_tail["i"] = i

            def pe_pin(i):
                if pe_tail["i"] is not None:
                    add_dep_helper(i.ins, pe_tail["i"].ins, sync=False,
                                   reason="pe order pin")
                pe_tail["i"] = i

            def emit_g(entry, is_last):
                # G matmuls for block b, emitted two blocks late so PE never
                # stalls on the sqrt->moh chain. The first matmul's DVE wait
                # (moh) transitively covers ACT's sqrt via vector clocks.
                b, i_moh = entry
                first = None
                for tt in range(TPB):
                    t = b * TPB + tt
                    i_g = nc.tensor.matmul(
                        out=G_ps[:], lhsT=ohmoh[:, t, :, :], rhs=dd[:, t, :],
                        start=(t == 0),
                        stop=(is_last and tt == TPB - 1),
                        skip_group_check=True)
                    if first is None:
                        first = i_g
                        add_dep_helper(i_g.ins, i_moh.ins, sync=True,
                                       reason="pe observes moh")
                    pe_pin(i_g)
                return first

            ohcasts = []

            def emit_cast(c):
                cast = nc.gpsimd.dma_start(
                    out=x8s[c],
                    in_=x8_in[:, c * TPC * 128:(c + 1) * TPC * 128])
                casts.append(cast)
                ocast = nc.gpsimd.dma_start(
                    out=ohmoh[:, c * TPC:(c + 1) * TPC, 0, :],
                    in_=oh_in[:, c * TPC * K:(c + 1) * TPC * K].rearrange(
                        "p (t k) -> p t k", k=K))
                ohcasts.append(ocast)

            for c in range(NCH):
                # prefetch: chunk c+1's DMAs are issued at chunk c's head so
                # the transfers overlap chunk c's compute.
                if c == 0:
                    emit_cast(0)
                    emit_cast(1)
                elif c < NCH - 1:
                    emit_cast(c + 1)

                # PE observes the chunk DMAs once, so in-chunk xc/G matmuls
                # never carry a DMA wait themselves.
                npA = nc.tensor.nop()
                add_dep_helper(npA.ins, casts[c].ins, sync=True,
                               reason="pe observes x chunk dma")
                pe_pin(npA)
                npA2 = nc.tensor.nop()
                add_dep_helper(npA2.ins, ohcasts[c].ins, sync=True,
                               reason="pe observes oh chunk dma")
                pe_pin(npA2)
                # DVE observes the oh chunk DMA (moh reads the oh half).
                npD = nc.vector.nop()
                add_dep_helper(npD.ins, ohcasts[c].ins, sync=True,
                               reason="dve observes oh chunk dma")
                dve_pin(npD)

                for bb in range(BPC):
                    b = c * BPC + bb

                    psum = pxc.tile([128, TPB * K], f32, tag="pxc")
                    # bias preinit: psum[row, t, j] = c2[j] + x2[row, t]
                    i_pre = nc.vector.tensor_tensor(
                        out=psum[:].rearrange("p (t k) -> p t k", k=K),
                        in0=c2blk.rearrange("p (t k) -> p t k", k=K),
                        in1=x2cols[:, b * TPB:(b + 1) * TPB, None].to_broadcast(
                            (128, TPB, K)),
                        op=mybir.AluOpType.add)
                    dve_pin(i_pre)

                    for tt in range(TPB):
                        i_mm = nc.tensor.matmul(
                            out=psum[:, tt * K:(tt + 1) * K],
                            lhsT=x8s[c][:, (bb * TPB + tt) * 128:
                                        (bb * TPB + tt) * 128 + 128],
                            rhs=ctf8[:],
                            start=False, stop=(tt == TPB - 1),
                            skip_group_check=True)
                        if tt == 0:
                            add_dep_helper(i_mm.ins, i_pre.ins, sync=True,
                                           reason="pe observes preinit")
                        pe_pin(i_mm)

                    i_sqrt = nc.scalar.activation(
                        out=dd[:, b * TPB:(b + 1) * TPB, :],
                        in_=psum[:].rearrange("p (t k) -> p t k", k=K),
                        func=mybir.ActivationFunctionType.Sqrt)
                    sqrts.append(i_sqrt)

                    i_moh = nc.vector.tensor_tensor(
                        out=ohmoh[:, b * TPB:(b + 1) * TPB, 1, :],
                        in0=dd[:, b * TPB:(b + 1) * TPB, :],
                        in1=ohmoh[:, b * TPB:(b + 1) * TPB, 0, :],
                        op=mybir.AluOpType.mult)
                    dve_pin(i_moh)

                    blocks.append((b, i_moh))
                    if b >= 2:
                        emit_g(blocks[b - 2], False)

            emit_g(blocks[NB - 2], False)
            i_g_last = emit_g(blocks[NB - 1], True)

            # ---- finale ----
            pn2 = nc.gpsimd.nop()
            add_dep_helper(pn2.ins, casts[-1].ins, sync=True,
                           reason="pool observes casts")
            G_sb = sb.tile([128, K], f32)
            i_gc = nc.vector.tensor_copy(out=G_sb, in_=G_ps[:])
            dve_pin(i_gc)
            e1 = nc.gpsimd.dma_start(out=out_G[:], in_=G_sb)
            add_dep_helper(e1.ins, pn2.ins, sync=False, reason="pin")
            for inst in [i_g_last, sqrts[-1], i_gc, e1, casts[-1]]:
                n = nc.sync.nop()
                add_dep_helper(n.ins, inst.ins, sync=True, reason="end chain")
    _strip_redundant_waits(nc)
    return nc


def _host_prep(centroids):
    c = np.ascontiguousarray(centroids, dtype=np.float32)          # [K, D]
    ct = (-2.0 * c).astype(E4M3)                                   # [K, D] fp8
    ct_f = ct.astype(np.float32)
    ctf8 = np.ascontiguousarray(ct.T)                              # [D, K] fp8
    # device-effective centroids: c_eff[d, j] = -ct[j, d]/2 (exact /2)
    c_eff = -0.5 * ct_f.T                                          # [D, K]
    c2t = (c_eff.astype(np.float64) ** 2).sum(0).astype(np.float32)  # [K]
    c2blk = np.tile(c2t, TPB).astype(ml_dtypes.bfloat16)           # [512] bf16
    c2_dev = c2blk.astype(np.float32)[:K]
    ctsum = ct_f.sum(0)                          # [D]: sum_j (-2 c_eff[., j])
    c2s = float(c2_dev.astype(np.float64).sum())
    return ctf8, c2blk, ctsum, c2s


def _pack_consts(ctf8, c2blk, x2cols):
    """Per-partition pack (bytes): ctf8[64] c2blk bf16[1024] x2cols f32[1024]
    -> [128, 528] f32 view."""
    def u8(a):
        return np.ascontiguousarray(a).view(np.uint8)
    c2row = u8(np.ascontiguousarray(c2blk))                        # [1024] u8
    rows = []
    for p in range(128):
        rows.append(np.concatenate([
            u8(ctf8[p]), c2row, u8(x2cols[p]),
        ]))
    buf = np.stack(rows)                          # [128, 2112] u8
    return np.ascontiguousarray(buf).view(np.float32)


def kernel(embeddings, cluster_labels, centroids):
    embeddings = np.ascontiguousarray(embeddings, dtype=np.float32)
    cluster_labels = np.ascontiguousarray(cluster_labels, dtype=np.int64)
    centroids = np.ascontiguousarray(centroids, dtype=np.float32)

    if "nc" not in _cache:
        _cache["nc"] = _build()
    nc = _cache["nc"]

    ctf8, c2blk, ctsum, c2s = _host_prep(centroids)

    x8_full = embeddings.astype(E4M3)                             # [N, D] fp8
    x2_full = (embeddings.astype(np.float64) ** 2).sum(1).astype(np.float32)

    oh_full = (cluster_labels[:, None] ==
               np.arange(K, dtype=np.int64)[None, :]).astype(np.float16)

    in_maps = []
    for cix in range(NCORES):
        sl = slice(cix * NS, (cix + 1) * NS)
        x8 = x8_full[sl]                                          # [NS, D]
        x2cols = np.ascontiguousarray(
            x2_full[sl].reshape(T, 128).T)                        # [128, T] f32
        # oh tile-major: ohp[p, t, k] = oh[t*128+p, k]
        ohp = np.ascontiguousarray(
            oh_full[sl].reshape(T, 128, K).transpose(1, 0, 2))    # [128, T, K]

        consts = _pack_consts(ctf8, c2blk, x2cols)
        in_maps.append({
            "x8": np.ascontiguousarray(x8.T),                     # [D, NS] fp8
            "ohf": ohp.reshape(128, T * K),
            "consts": consts,
        })
    res = run_bass_kernel_spmd(nc, in_maps, core_ids=list(range(NCORES)))
    _cache["last_res"] = res

    Gt = np.zeros((128, K), np.float64)
    for r in res.results:
        Gt += r["out_G"].astype(np.float64)
    G1 = Gt[0:K, :].T                             # [j, k]
    G2 = Gt[K:2 * K, :].T

    labels = cluster_labels
    counts = np.bincount(labels, minlength=K).astype(np.float64)
    O = np.diag(G1)                               # sum_{n in k} dist_own
    S1 = G1.sum(0)                                # sum_{n in k} sum_j dist
    W = S1 - O
    A = np.diag(G2)                               # sum_{n in k} d2_own

    # closed-form S2_k = sum_{n in k} sum_j d2[n, j], device-consistent
    q = x8_full.astype(np.float32) @ ctsum        # [N]: -2 x.(sum_j c_j)
    sx2 = np.bincount(labels, weights=x2_full.astype(np.float64), minlength=K)
    sq = np.bincount(labels, weights=q.astype(np.float64), minlength=K)
    S2 = K * sx2 + counts * c2s + sq
    V = S2 - A

    safe = np.maximum(counts, 1.0)
    t_k = A + (MARGIN * MARGIN * (K - 1) * counts - 2.0 * MARGIN * W + V) / (K - 1)
    loss = np.where(counts > 0, t_k / safe, 0.0).sum() / K
    return np.float32(loss)


# revision 22
# speedup vs baseline: 1.6165x; 1.6165x over previous
"""DistanceCentroidLoss on 8 Trainium2 NeuronCores (Bass/Tile).

Data-parallel over N: each core processes 32768 rows. The host pre-transposes
x to xT [D, NS] bf16 (tile-major row mapping n = t*128 + r) so the device
needs no DMA transposes and HBM traffic is halved. Per-core device work:
  d2[n,j] = |x_n|^2 + |c_j|^2 - 2 x.c_j   (bf16 inputs, f32 PSUM accumulation)
  dist    = sqrt(d2)
  G[0:64,  k] = sum_n dist[n,j] * onehot[n,k]   (PE segment matmuls)
  G[64:128,k] = sum_n d2[n,j]   * onehot[n,k]
  cnt[t,k]    = sum_n onehot[n,k]
The tiny cross-core/cluster reduction to the scalar loss runs on host:
  A_k = G_d2[k,k],  V_k = colsum(G_d2)_k - A_k,  W_k = colsum(G_dist)_k - G_dist[k,k]
  loss = (1/K) sum_k [ A_k + (M^2*63*c_k - 2M*W_k + V_k)/63 ] / max(c_k,1)

Toolchain quirk: this walrus build rejects any instruction with more than one
semaphore wait. The kernel is structured so every instruction needs at most
one cross-proc wait (single SWDGE completion lane, persistent arenas,
wait-carrying instruction ordering), plus a conservative post-pass that strips
semantically-redundant waits Tile emits (own-stream tautologies and
already-observed sem values). Note Tile also serializes successive dynamic
DMAs against each other's completion (descriptor-ring reuse on the single
lane), so the kernel keeps all data movement on one SWDGE stream: packed
constants first, then the 8 x-chunk loads back-to-back.
"""
import sys

sys.path.insert(0, '/opt/trn_rl_repo')

import numpy as np
import ml_dtypes

import concourse.bass as bass
import concourse.mybir as mybir
import concourse.tile_sem_assignment as _tsa
from concourse.tile import TileContext, add_dep_helper
from concourse.bass_utils import run_bass_kernel_spmd

f32 = mybir.dt.float32
bf16 = mybir.dt.bfloat16
f16 = mybir.dt.float16
i32 = mybir.dt.int32

N, D, K = 262144, 128, 64
NCORES = 8
NS = N // NCORES        # rows per core = 32768
T = NS // 128           # 256 tiles of 128 rows
TPB = 8                 # tiles per block
NB = T // TPB           # 32 blocks
TPC = 32                # tiles per DMA chunk
NCH = T // TPC          # 8 chunks
BPC = TPC // TPB        # blocks per chunk = 4
MARGIN = 5.0
CW = 960                # const pack: f32 words per partition

_cache = {}

_OWN_PREFIX = {
    "Activation": ("Activation_",),
    "DVE": ("DVE_",),
    "PE": ("PE_",),
    "Pool": ("Pool_",),
    "SP": ("SP_sequencer",),
}


def _is_own(eng, name):
    for p in _OWN_PREFIX.get(eng, ()):
        if name.startswith(p) and not name.startswith("barrier"):
            return True
    return False


def _strip_redundant_waits(nc):
    """Drop tautological sem waits (see module docstring)."""
    own, seen = {}, {}
    for blk in nc.m.functions[0].blocks:
        for inst in blk.instructions:
            eng = str(inst.engine).split(".")[-1]
            si = inst.sync_info
            oc = own.setdefault(eng, {})
            ob = seen.setdefault(eng, {})
            if si is not None and si.on_wait:
                keep = []
                for w in si.on_wait:
                    if w.sync_type != "semaphore" or w.wait_mode != "sem-ge-imm" \
                            or w.wait_reg is not None or w.wait_value is None:
                        keep.append(w)
                        continue
                    nm, v = w.ant_name, w.wait_value
                    if "barrier" in nm:
                        # barrier EVSEMs are sem-sub'd (non-monotonic): never elide
                        keep.append(w)
                        continue
                    if ob.get(nm, -1) >= v or (_is_own(eng, nm) and oc.get(nm, 0) >= v):
                        continue
                    keep.append(w)
                    ob[nm] = max(ob.get(nm, -1), v)
                if len(keep) != len(si.on_wait):
                    si.on_wait = keep
                    inst.sync_info = si
            if si is not None and si.on_update:
                dma = "DMA" in type(inst).__name__ or "DmaTranspose" in type(inst).__name__
                for u in si.on_update:
                    if u.update_mode != "sem-inc" or u.update_value is None:
                        continue
                    if not dma and _is_own(eng, u.ant_name):
                        oc[u.ant_name] = oc.get(u.ant_name, 0) + u.update_value
    return nc


def _build():
    # single completion-sem lane for SWDGE: all DMAs share DMASW0, so
    # consumers never carry waits on more than one DMA proc.
    _tsa.NUM_SWDGE_GLOBAL_SEMS = 1
    _tsa.NUM_HWDGE_SEMS = 1

    nc = bass.Bass()
    xT_in = nc.dram_tensor("xT", [D, NS], bf16, kind="ExternalInput")
    consts_in = nc.dram_tensor("consts", [128, CW], f32, kind="ExternalInput")
    out_G = nc.dram_tensor("out_G", [128, K], f32, kind="ExternalOutput")

    with TileContext(nc) as tc:
        with tc.tile_pool(name="single", bufs=1) as sb, \
             tc.tile_pool(name="xsqp", bufs=2) as xsqp, \
             tc.tile_pool(name="ohp", bufs=2) as ohp, \
             tc.tile_pool(name="pxc", bufs=2, space="PSUM") as pxc, \
             tc.tile_pool(name="pacc", bufs=1, space="PSUM") as pacc:

            # ---- one packed constant DMA on the SWDGE lane ----
            consts_sb = sb.tile([128, CW], f32)
            nc.gpsimd.dma_start(out=consts_sb, in_=consts_in[:])
            c2b_sb = consts_sb[:, 0:512]                       # [128,512] f32
            iota_tbk = consts_sb[:, 512:768].bitcast(f16).rearrange(
                "p (t k) -> p t k", k=K)                       # [128,8,64] f16
            labf_sb = consts_sb[:, 768:896].bitcast(f16)       # [128,256] f16
            cTm2_sb = consts_sb[:, 896:928].bitcast(bf16)      # [128,64] bf16
            ones64_sb = consts_sb[:, 928:960].bitcast(f16)     # [128,64] f16

            # c2 as a [1,512] bf16 row for the rank-1 PE add; doubles as the
            # early DVE observation of the consts DMA (so later DVE reads of
            # consts have their SWDGE wait elided)
            c2row = sb.tile([1, TPB * K], bf16)
            i_c2r = nc.vector.tensor_copy(out=c2row, in_=c2b_sb[0:1, :])
            ones1p = sb.tile([1, 128], bf16)
            nc.vector.memset(ones1p, 1.0)

            # ---- persistent per-chunk xT arenas ----
            xTs = [sb.tile([128, TPC, 128], bf16, name=f"xT{c}") for c in range(NCH)]
            dd = sb.tile([128, T, 2, K], f16)         # 8 MB: [dist | d2]

            G_ps = pacc.tile([128, K], f32)

            casts, end_list = [], []
            prev = None          # (dd tiles, oh, d2copy) of previous block
            dve_tail = {"i": None}

            def dve_pin(i):
                # Tile's lowering reorders ready instructions within an engine
                # queue; the wait-elision scheme depends on DVE executing in
                # emission order, so chain every DVE op with nosync pins.
                if dve_tail["i"] is not None:
                    add_dep_helper(i.ins, dve_tail["i"].ins, sync=False,
                                   reason="dve order pin")
                dve_tail["i"] = i

            dve_tail["i"] = i_c2r

            def emit_g(pb, is_last):
                # G matmuls for block pb, emitted one block late so PE never
                # stalls on same-block ACT. ptn pre-observes ACT's d2 copy
                # (covers the sqrt too, and the psum WAR of block pb+2).
                dd_b, oh_b, d2c_b = prev
                ptn = nc.tensor.nop()
                add_dep_helper(ptn.ins, d2c_b.ins, sync=True,
                               reason="pe observes act d2copy")
                i_g = None
                for tt in range(TPB):
                    t = pb * TPB + tt
                    i_g = nc.tensor.matmul(
                        out=G_ps, lhsT=dd[:, t, :, :], rhs=oh_b[:, tt, :],
                        start=(pb == 0 and tt == 0),
                        stop=(is_last and tt == TPB - 1),
                        skip_group_check=True)
                    if tt == 0:
                        add_dep_helper(i_g.ins, ptn.ins, sync=False, reason="pin")
                return i_g

            for c in range(NCH):
                cast = nc.gpsimd.dma_start(
                    out=xTs[c],
                    in_=xT_in[:, c * TPC * 128:(c + 1) * TPC * 128].rearrange(
                        "d (t r) -> d t r", r=128))
                casts.append(cast)

                for bb in range(BPC):
                    b = c * BPC + bb
                    xsq = xsqp.tile([128, TPB, 128], f16, tag="xsq")
                    xv = xTs[c][:, bb * TPB:(bb + 1) * TPB, :]
                    i_sq = nc.vector.tensor_mul(out=xsq, in0=xv, in1=xv)
                    dve_pin(i_sq)

                    psum_xc = pxc.tile([128, TPB * K], f32, tag="pxc")
                    for tt in range(TPB):
                        nc.tensor.matmul(out=psum_xc[:, tt * K:(tt + 1) * K],
                                         lhsT=xTs[c][:, bb * TPB + tt, :], rhs=cTm2_sb,
                                         start=True, stop=False,
                                         skip_group_check=True)
                    for tt in range(TPB):
                        nc.tensor.matmul(out=psum_xc[:, tt * K:(tt + 1) * K],
                                         lhsT=xsq[:, tt, :], rhs=ones64_sb,
                                         start=False, stop=False,
                                         skip_group_check=True)
                    i_c2 = nc.tensor.matmul(out=psum_xc[:],
                                            lhsT=ones1p, rhs=c2row,
                                            start=False, stop=True,
                                            skip_group_check=True)

                    i_sqrt = nc.scalar.activation(
                        out=dd[:, b * TPB:(b + 1) * TPB, 0, :],
                        in_=psum_xc[:].rearrange("p (t k) -> p t k", k=K),
                        func=mybir.ActivationFunctionType.Sqrt)
                    i_d2c = nc.scalar.copy(
                        out=dd[:, b * TPB:(b + 1) * TPB, 1, :],
                        in_=psum_xc[:].rearrange("p (t k) -> p t k", k=K))
                    add_dep_helper(i_d2c.ins, i_sqrt.ins, sync=False,
                                   reason="act order pin")

                    oh = ohp.tile([128, TPB, K], f16, tag="oh")
                    i_oh = nc.vector.tensor_tensor(
                        out=oh[:],
                        in0=iota_tbk,
                        in1=labf_sb[:, b * TPB:(b + 1) * TPB, None].to_broadcast(
                            (128, TPB, K)),
                        op=mybir.AluOpType.is_equal)
                    dve_pin(i_oh)

                    if prev is not None:
                        emit_g(b - 1, False)
                    prev = (dd, oh, i_d2c)
                    if b == NB - 1:
                        i_g = emit_g(b, True)
                        end_list += [i_g, i_sqrt, i_d2c, i_oh, i_sq]

            # ---- finale ----
            pn2 = nc.gpsimd.nop()
            add_dep_helper(pn2.ins, casts[-1].ins, sync=True, reason="pool observes casts")
            G_sb = sb.tile([128, K], f32)
            i_gc = nc.vector.tensor_copy(out=G_sb, in_=G_ps)
            e1 = nc.gpsimd.dma_start(out=out_G[:], in_=G_sb)
            add_dep_helper(e1.ins, pn2.ins, sync=False, reason="pin")
            for inst in end_list + [i_gc, e1, casts[-1]]:
                n = nc.sync.nop()
                add_dep_helper(n.ins, inst.ins, sync=True, reason="end chain")
    _strip_redundant_waits(nc)
    return nc


def _host_prep(centroids):
    c_bf = centroids.astype(ml_dtypes.bfloat16)
    c_f = c_bf.astype(np.float32)
    cTm2 = np.ascontiguousarray((-2.0 * c_f).T).astype(ml_dtypes.bfloat16)  # [D,K]
    c2 = (c_f.astype(np.float64) ** 2).sum(1).astype(np.float32)
    c2b8 = np.broadcast_to(np.tile(c2, TPB), (128, TPB * K)).astype(np.float32)
    iota = np.broadcast_to(np.tile(np.arange(K, dtype=np.float16), TPB),
                           (128, TPB * K))
    ones64 = np.ones((D, K), np.float16)
    return cTm2, c2b8, iota, ones64


def _pack_consts(c2b8, iota, labf, cTm2, ones64):
    """Pack per-partition: c2b[512 f32] iota[512 f16] labf[256 f16]
    cTm2[64 bf16] ones64[64 f16] -> [128, 960] f32 view."""
    def u8(a):
        return np.ascontiguousarray(a).view(np.uint8)
    rows = []
    for p in range(128):
        rows.append(np.concatenate([
            u8(c2b8[p]), u8(iota[p]), u8(labf[p]), u8(cTm2[p]), u8(ones64[p]),
        ]))
    buf = np.stack(rows)                      # [128, 3840] u8
    return np.ascontiguousarray(buf).view(np.float32)


def kernel(embeddings, cluster_labels, centroids):
    embeddings = np.ascontiguousarray(embeddings, dtype=np.float32)
    cluster_labels = np.ascontiguousarray(cluster_labels, dtype=np.int64)
    centroids = np.ascontiguousarray(centroids, dtype=np.float32)

    if "nc" not in _cache:
        _cache["nc"] = _build()
    nc = _cache["nc"]

    cTm2, c2b8, iota, ones64 = _host_prep(centroids)
    x_bf = embeddings.astype(ml_dtypes.bfloat16)
    in_maps = []
    for cix in range(NCORES):
        xs = x_bf[cix * NS:(cix + 1) * NS]                    # [NS, D]
        ls = cluster_labels[cix * NS:(cix + 1) * NS]
        labf = np.ascontiguousarray(
            ls.reshape(T, 128).T).astype(np.float16)          # [128, T]
        # cTm2 is [D, K]; per-partition row p of the pack carries cTm2[p]
        consts = _pack_consts(c2b8, iota, labf, cTm2, ones64)
        in_maps.append({
            "xT": np.ascontiguousarray(xs.T),                 # [D, NS] bf16
            "consts": consts,
        })
    res = run_bass_kernel_spmd(nc, in_maps, core_ids=list(range(NCORES)))
    _cache["last_res"] = res

    G = np.zeros((128, K), np.float64)
    for r in res.results:
        G += r["out_G"].astype(np.float64)
    counts = np.bincount(cluster_labels, minlength=K).astype(np.float64)
    G_dist = G[:K, :]
    G_d2 = G[K:, :]
    A = np.diag(G_d2)
    V = G_d2.sum(0) - A
    W = G_dist.sum(0) - np.diag(G_dist)
    safe = np.maximum(counts, 1.0)
    t_k = A + (MARGIN * MARGIN * (K - 1) * counts - 2.0 * MARGIN * W + V) / (K - 1)
    loss = np.where(counts > 0, t_k / safe, 0.0).sum() / K
    return np.float32(loss)



# revision 23
# speedup vs baseline: 1.8537x; 1.1467x over previous
"""DistanceCentroidLoss on 8 Trainium2 NeuronCores (Bass/Tile), v7.

Per 128-row tile, ONE fp8 DoubleRow matmul computes the full quantized
d2[n,j] = x2[n] + c2[j] - 2 x.c_j into PSUM (x/cT packed [64,2,*] fp8;
partitions 64/65 carry a two-digit fp8 decomposition of the x2/c2 bias --
verified path). ACT sqrts PSUM -> dist f16 (dd). DVE builds oh and
moh = dist*oh. PE accumulates G_t[s*64+k, j] += sum_n ohm[n,s,k]*dist[n,j]
with lhsT = ohm [128, 2, 64] (fast 128-column ldweights) and rhs = dd tile.
G is emitted two blocks late and FIRST in each PE slot. Chunk DMAs are
prefetched one chunk ahead. Host assembles the scalar loss from G's
diagonals/colsums plus a closed-form V (device-consistent fp8 values).
"""
import sys

sys.path.insert(0, '/opt/trn_rl_repo')

import numpy as np
import ml_dtypes

import concourse.bass as bass
import concourse.mybir as mybir
import concourse.tile_sem_assignment as _tsa
from concourse.tile import TileContext, add_dep_helper
from concourse.bass_utils import run_bass_kernel_spmd

f32 = mybir.dt.float32
bf16 = mybir.dt.bfloat16
f16 = mybir.dt.float16
f8 = mybir.dt.float8e4
DR = mybir.MatmulPerfMode.DoubleRow
E4M3 = ml_dtypes.float8_e4m3

N, D, K = 262144, 128, 64
NCORES = 8
NS = N // NCORES
T = NS // 128
TPB = 8
NB = T // TPB
TPC = 32
NCH = T // TPC
BPC = TPC // TPB
MARGIN = 5.0
CW = 416

_cache = {}

_OWN_PREFIX = {
    "Activation": ("Activation_",),
    "DVE": ("DVE_",),
    "PE": ("PE_",),
    "Pool": ("Pool_",),
    "SP": ("SP_sequencer",),
}


def _is_own(eng, name):
    for p in _OWN_PREFIX.get(eng, ()):
        if name.startswith(p) and not name.startswith("barrier"):
            return True
    return False


def _strip_redundant_waits(nc):
    """Drop tautological sem waits (baseline-proven simple pass)."""
    own, seen = {}, {}
    for blk in nc.m.functions[0].blocks:
        for inst in blk.instructions:
            eng = str(inst.engine).split(".")[-1]
            si = inst.sync_info
            oc = own.setdefault(eng, {})
            ob = seen.setdefault(eng, {})
            if si is not None and si.on_wait:
                keep = []
                for w in si.on_wait:
                    if w.sync_type != "semaphore" or w.wait_mode != "sem-ge-imm" \
                            or w.wait_reg is not None or w.wait_value is None:
                        keep.append(w)
                        continue
                    nm, v = w.ant_name, w.wait_value
                    if "barrier" in nm:
                        keep.append(w)
                        continue
                    if ob.get(nm, -1) >= v or (_is_own(eng, nm) and oc.get(nm, 0) >= v):
                        continue
                    keep.append(w)
                    ob[nm] = max(ob.get(nm, -1), v)
                if len(keep) != len(si.on_wait):
                    si.on_wait = keep
                    inst.sync_info = si
            if si is not None and si.on_update:
                dma = "DMA" in type(inst).__name__ or "DmaTranspose" in type(inst).__name__
                for u in si.on_update:
                    if u.update_mode != "sem-inc" or u.update_value is None:
                        continue
                    if not dma and _is_own(eng, u.ant_name):
                        oc[u.ant_name] = oc.get(u.ant_name, 0) + u.update_value
    return nc


def _build():
    _tsa.NUM_SWDGE_GLOBAL_SEMS = 1
    _tsa.NUM_HWDGE_SEMS = 1

    nc = bass.Bass()
    x8_in = nc.dram_tensor("x8", [66, T * 2 * 128], f8, kind="ExternalInput")
    consts_in = nc.dram_tensor("consts", [128, CW], f32, kind="ExternalInput")
    out_G = nc.dram_tensor("out_G", [128, K], f32, kind="ExternalOutput")

    with TileContext(nc) as tc:
        with tc.tile_pool(name="single", bufs=1) as sb, \
             tc.tile_pool(name="ohp", bufs=3) as ohp, \
             tc.tile_pool(name="pxc", bufs=2, space="PSUM") as pxc, \
             tc.tile_pool(name="pacc", bufs=1, space="PSUM") as pacc:

            consts_sb = sb.tile([128, CW], f32)
            cdma = nc.gpsimd.dma_start(out=consts_sb, in_=consts_in[:])
            iota_tbk = consts_sb[:, 0:256].bitcast(f16).rearrange(
                "p (t k) -> p t k", k=K)
            labf_sb = consts_sb[:, 256:384].bitcast(f16)
            ctf8 = consts_sb[:, 384:416].bitcast(f8).rearrange(
                "p (q j) -> p q j", j=K)

            x8s = [sb.tile([66, TPC, 2, 128], f8, name=f"x8{c}") for c in range(NCH)]
            dd = sb.tile([128, T, K], f16)

            G_ps = pacc.tile([128, K], f32)

            casts = []
            sqrts = []
            blocks = []
            dve_tail = {"i": None}
            pe_tail = {"i": None}

            def dve_pin(i):
                if dve_tail["i"] is not None:
                    add_dep_helper(i.ins, dve_tail["i"].ins, sync=False,
                                   reason="dve order pin")
                dve_tail["i"] = i

            def pe_pin(i):
                if pe_tail["i"] is not None:
                    add_dep_helper(i.ins, pe_tail["i"].ins, sync=False,
                                   reason="pe order pin")
                pe_tail["i"] = i

            def emit_g(entry, is_last):
                # two blocks late; npB pre-observes DVE's moh (covers ACT's
                # sqrt transitively and the psum bank WAR for following xc)
                b, ohm_b, i_moh = entry
                npB = nc.tensor.nop()
                add_dep_helper(npB.ins, i_moh.ins, sync=True,
                               reason="pe observes moh")
                pe_pin(npB)
                for tt in range(TPB):
                    t = b * TPB + tt
                    i_g = nc.tensor.matmul(
                        out=G_ps[:], lhsT=ohm_b[:, tt, :, :], rhs=dd[:, t, :],
                        start=(t == 0),
                        stop=(is_last and tt == TPB - 1),
                        skip_group_check=True)
                    pe_pin(i_g)
                return i_g

            def emit_cast(c):
                cast = nc.gpsimd.dma_start(
                    out=x8s[c],
                    in_=x8_in[:, c * TPC * 256:(c + 1) * TPC * 256].rearrange(
                        "d (t q m) -> d t q m", q=2, m=128))
                casts.append(cast)

            for c in range(NCH):
                if c == 0:
                    emit_cast(0)
                    emit_cast(1)
                elif c < NCH - 1:
                    emit_cast(c + 1)

                npA = nc.tensor.nop()
                add_dep_helper(npA.ins, casts[c].ins, sync=True,
                               reason="pe observes chunk dma")
                pe_pin(npA)

                for bb in range(BPC):
                    b = c * BPC + bb
                    if b >= 2:
                        emit_g(blocks[b - 2], False)

                    # npB of the G above covered the bank WAR (moh(b-2) waited
                    # on sqrt(b-2)); for b<2 the bank is fresh.
                    psum = pxc.tile([128, TPB * K], f32, tag="pxc")
                    for tt in range(TPB):
                        i_mm = nc.tensor.matmul(
                            out=psum[:, tt * K:(tt + 1) * K],
                            lhsT=x8s[c][:, bb * TPB + tt, :, :],
                            rhs=ctf8[0:66, :, :],
                            start=True, stop=True, perf_mode=DR,
                            skip_group_check=True)
                        pe_pin(i_mm)

                    i_sqrt = nc.scalar.activation(
                        out=dd[:, b * TPB:(b + 1) * TPB, :],
                        in_=psum[:].rearrange("p (t k) -> p t k", k=K),
                        func=mybir.ActivationFunctionType.Sqrt)
                    sqrts.append(i_sqrt)

                    ohm = ohp.tile([128, TPB, 2, K], f16, tag="ohm")
                    i_oh = nc.vector.tensor_tensor(
                        out=ohm[:, :, 0, :],
                        in0=iota_tbk,
                        in1=labf_sb[:, b * TPB:(b + 1) * TPB, None].to_broadcast(
                            (128, TPB, K)),
                        op=mybir.AluOpType.is_equal)
                    dve_pin(i_oh)
                    i_moh = nc.vector.tensor_tensor(
                        out=ohm[:, :, 1, :],
                        in0=dd[:, b * TPB:(b + 1) * TPB, :],
                        in1=ohm[:, :, 0, :],
                        op=mybir.AluOpType.mult)
                    dve_pin(i_moh)

                    blocks.append((b, ohm, i_moh))

            emit_g(blocks[NB - 2], False)
            i_g_last = emit_g(blocks[NB - 1], True)

            pn2 = nc.gpsimd.nop()
            add_dep_helper(pn2.ins, casts[-1].ins, sync=True,
                           reason="pool observes casts")
            G_sb = sb.tile([128, K], f32)
            i_gc = nc.vector.tensor_copy(out=G_sb, in_=G_ps[:])
            dve_pin(i_gc)
            e1 = nc.gpsimd.dma_start(out=out_G[:], in_=G_sb)
            add_dep_helper(e1.ins, pn2.ins, sync=False, reason="pin")
            for inst in [i_g_last, sqrts[-1], i_gc, e1, casts[-1]]:
                n = nc.sync.nop()
                add_dep_helper(n.ins, inst.ins, sync=True, reason="end chain")
    _strip_redundant_waits(nc)
    return nc


def _two_digit(v):
    a = np.clip(np.floor(v / 16.0 + 0.5), 0.0, 14.0).astype(np.float32)
    hi = 16.0 * a
    lo = (v - hi).astype(np.float32).astype(E4M3)
    return hi, lo


def _host_prep(centroids):
    c = np.ascontiguousarray(centroids, dtype=np.float32)
    ct = (-2.0 * c).astype(E4M3)                                   # [K, D]
    ct_f = ct.astype(np.float32)
    c_eff = -0.5 * ct_f.T                                          # [D, K]
    c2t = (c_eff.astype(np.float64) ** 2).sum(0).astype(np.float32)
    c2hi, c2lo8 = _two_digit(c2t)
    c2_dev = c2hi + c2lo8.astype(np.float32)

    ctf8 = np.zeros((128, 2, K), dtype=E4M3)
    ctf8[0:64] = ct.T.reshape(2, 64, K).transpose(1, 0, 2)
    ctf8[64, 0, :] = E4M3(1.0)
    ctf8[64, 1, :] = c2hi.astype(E4M3)
    ctf8[65, 0, :] = c2lo8
    ctf8[65, 1, :] = E4M3(1.0)

    ctsum = ct_f.sum(0)
    c2s = float(c2_dev.astype(np.float64).sum())
    return ctf8, ctsum, c2s


def _pack_consts(labf, ctf8):
    iota = np.tile(np.arange(K, dtype=np.float16), TPB)
    iota_b = np.broadcast_to(iota, (128, TPB * K))

    def u8(a):
        return np.ascontiguousarray(a).view(np.uint8)
    ct_rows = np.ascontiguousarray(ctf8.reshape(128, 2 * K)).view(np.uint8)
    rows = []
    for p in range(128):
        rows.append(np.concatenate([u8(iota_b[p]), u8(labf[p]), ct_rows[p]]))
    buf = np.stack(rows)
    return np.ascontiguousarray(buf).view(np.float32)


def kernel(embeddings, cluster_labels, centroids):
    embeddings = np.ascontiguousarray(embeddings, dtype=np.float32)
    cluster_labels = np.ascontiguousarray(cluster_labels, dtype=np.int64)
    centroids = np.ascontiguousarray(centroids, dtype=np.float32)

    if "nc" not in _cache:
        _cache["nc"] = _build()
    nc = _cache["nc"]

    ctf8, ctsum, c2s = _host_prep(centroids)

    x8_full = embeddings.astype(E4M3)
    x2_full = (embeddings.astype(np.float64) ** 2).sum(1).astype(np.float32)
    x2hi, x2lo8 = _two_digit(x2_full)
    x2_dev = x2hi + x2lo8.astype(np.float32)

    in_maps = []
    for cix in range(NCORES):
        sl = slice(cix * NS, (cix + 1) * NS)
        x8 = x8_full[sl]
        ls = cluster_labels[sl]
        labf = np.ascontiguousarray(
            ls.reshape(T, 128).T).astype(np.float16)

        arena = np.zeros((66, T, 2, 128), dtype=E4M3)
        arena[0:64] = x8.reshape(T, 128, 2, 64).transpose(3, 0, 2, 1)
        arena[64, :, 0, :] = x2hi[sl].astype(E4M3).reshape(T, 128)
        arena[64, :, 1, :] = E4M3(1.0)
        arena[65, :, 0, :] = E4M3(1.0)
        arena[65, :, 1, :] = x2lo8[sl].reshape(T, 128)

        consts = _pack_consts(labf, ctf8)
        in_maps.append({
            "x8": np.ascontiguousarray(arena.reshape(66, T * 2 * 128)),
            "consts": consts,
        })
    res = run_bass_kernel_spmd(nc, in_maps, core_ids=list(range(NCORES)))
    _cache["last_res"] = res

    Gt = np.zeros((128, K), np.float64)
    for r in res.results:
        Gt += r["out_G"].astype(np.float64)
    G1 = Gt[0:K, :].T
    G2 = Gt[K:2 * K, :].T

    labels = cluster_labels
    counts = np.bincount(labels, minlength=K).astype(np.float64)
    O = np.diag(G1)
    S1 = G1.sum(0)
    W = S1 - O
    A = np.diag(G2)

    q = x8_full.astype(np.float32) @ ctsum
    sx2 = np.bincount(labels, weights=x2_dev.astype(np.float64), minlength=K)
    sq = np.bincount(labels, weights=q.astype(np.float64), minlength=K)
    S2 = K * sx2 + counts * c2s + sq
    V = S2 - A

    safe = np.maximum(counts, 1.0)
    t_k = A + (MARGIN * MARGIN * (K - 1) * counts - 2.0 * MARGIN * W + V) / (K - 1)
    loss = np.where(counts > 0, t_k / safe, 0.0).sum() / K
    return np.float32(loss)
